# revision 7
# baseline (speedup 1.0000x reference)
"""Trainium2 Bass kernel for nn_FGateEncoder (LSTM-style recurrence with field gate).

Strategy: data-parallel over batch across 8 NeuronCores (8 sequences per core,
weights replicated). Per core:
  1. Gather wte/wfe/wpe embedding rows on-device via indirect DMA
     (token-major tiles [128 tokens, 512]).
  2. PE-transpose to [H-on-partitions, tokens] and run the big time-independent
     matmuls: gx = x @ wx.T + b  and  zf = field_pos @ wf.T + bf, then
     lz = sigmoid(zf[:H]) * tanh(zf[H:]).  Results spill to DRAM scratch in a
     time-major layout the scan can stream.
  3. Sequential 512-step scan, everything laid out [feature-on-partitions,
     batch(8) on free dim]: per step 64 accumulating matmuls
     (g[128gate,8b] += whT_tile.T @ h_chunk), sigmoid/tanh on the scalar
     engine, c/h updates on the vector engine.  No transposes on the critical
     path; h chunks feed the next step's matmuls directly.
Host only reorders/slices inputs and reassembles outputs.
"""

import os

import numpy as np

import concourse.bass as bass
import concourse.mybir as mybir
import concourse.tile as tile
from concourse.bass_utils import run_bass_kernel_spmd
from concourse.masks import make_identity

F32 = mybir.dt.float32
I32 = mybir.dt.int32
AF = mybir.ActivationFunctionType

B, S, H = 64, 512, 512
V, FV, PV = 50257, 2000, 1024
NCORES = 8
BL = B // NCORES          # 8 sequences per core
NTOK = S * BL             # 4096 tokens per core
NTT = NTOK // 128         # 32 token tiles
CHUNK_TT = 4              # token tiles per precompute chunk (512 tokens)
NCHUNK = NTT // CHUNK_TT  # 8
SCAN_CH = 16              # scan steps per gx/lz stream chunk
G4 = 4 * H                # 2048 gate dims
NGT = G4 // 128           # 16 gate tiles
NZT = 2 * H // 128        # 8 zf tiles
NKT = H // 128            # 4 contraction tiles

_STEPS = int(os.environ.get("FGATE_DEV_STEPS", str(S)))


def _split_multiwaits(nc):
    """walrus in this container rejects >1 sync-wait per instruction; Tile's
    kernel-tail drain accumulates several.  Split extras into standalone
    sequencer waits placed immediately before the offending instruction."""
    n = 0
    for f in nc.m.functions:
        for blk in f.blocks:
            out = []
            for inst in blk.instructions:
                si = inst.sync_info
                if si is not None and si.on_wait is not None and len(si.on_wait) > 1:
                    waits = list(si.on_wait)
                    for j, w in enumerate(waits[:-1]):
                        out.append(
                            mybir.InstEventSemaphore(
                                name=f"{inst.name}_splitwait_{j}",
                                engine=inst.engine,
                                ins=[],
                                outs=[],
                                sync_info=mybir.SyncInfo(on_wait=[w], on_update=[]),
                            )
                        )
                        n += 1
                    inst.sync_info = mybir.SyncInfo(
                        on_wait=[waits[-1]], on_update=si.on_update
                    )
                out.append(inst)
            blk.instructions = out
    return n


def _build(steps=_STEPS):
    nc = bass.Bass("TRN2", target_bir_lowering=False, debug=False, num_devices=NCORES)

    # ---- I/O ----
    idx_x = nc.dram_tensor("idx_x", [128, NTT], I32, kind="ExternalInput")
    idx_f = nc.dram_tensor("idx_f", [128, NTT], I32, kind="ExternalInput")
    idx_p = nc.dram_tensor("idx_p", [128, NTT], I32, kind="ExternalInput")
    idx_r = nc.dram_tensor("idx_r", [128, NTT], I32, kind="ExternalInput")
    wte = nc.dram_tensor("wte", [V, H], F32, kind="ExternalInput")
    wfe = nc.dram_tensor("wfe", [FV, H], F32, kind="ExternalInput")
    wpe = nc.dram_tensor("wpe", [PV, H], F32, kind="ExternalInput")
    wxT = nc.dram_tensor("wxT", [H, G4], F32, kind="ExternalInput")
    whT = nc.dram_tensor("whT", [H, G4], F32, kind="ExternalInput")
    wfT = nc.dram_tensor("wfT", [H, 2 * H], F32, kind="ExternalInput")
    bg = nc.dram_tensor("bg", [128, NGT], F32, kind="ExternalInput")
    bfg = nc.dram_tensor("bfg", [128, NZT], F32, kind="ExternalInput")

    fp_out = nc.dram_tensor("fp_out", [NTOK, H], F32, kind="ExternalOutput")
    hs_out = nc.dram_tensor("hs_out", [128, S * 32], F32, kind="ExternalOutput")
    c_out = nc.dram_tensor("c_out", [128, 32], F32, kind="ExternalOutput")

    # DRAM scratch, time-major for the scan:
    #   gx_d col layout: gi*4096 + t*8 + b      (per partition p = gate 128*gi+p)
    #   lz_d col layout: j*4096 + t*8 + b       (per partition p = h-dim 128*j+p)
    gx_d = nc.dram_tensor("gx_scratch", [128, NGT * NTOK], F32)
    lz_d = nc.dram_tensor("lz_scratch", [128, NKT * NTOK], F32)

    with tile.TileContext(nc) as tc:
        with (
            tc.tile_pool(name="const", bufs=1) as constp,
            tc.tile_pool(name="weights", bufs=1) as wpool,
        ):
            ident = constp.tile([128, 128], F32)
            make_identity(nc, ident[:])
            bg_sb = constp.tile([128, NGT], F32)
            nc.sync.dma_start(bg_sb, bg.ap())
            bfg_sb = constp.tile([128, NZT], F32)
            nc.sync.dma_start(bfg_sb, bfg.ap())
            ix_sb = constp.tile([128, NTT], I32)
            nc.sync.dma_start(ix_sb, idx_x.ap())
            if_sb = constp.tile([128, NTT], I32)
            nc.sync.dma_start(if_sb, idx_f.ap())
            ip_sb = constp.tile([128, NTT], I32)
            nc.sync.dma_start(ip_sb, idx_p.ap())
            ir_sb = constp.tile([128, NTT], I32)
            nc.sync.dma_start(ir_sb, idx_r.ap())

            wx_sb = [wpool.tile([128, G4], F32, tag=f"wx{k}", name=f"wx{k}") for k in range(NKT)]
            wf_sb = [wpool.tile([128, 2 * H], F32, tag=f"wf{k}", name=f"wfw{k}") for k in range(NKT)]
            wh_sb = [wpool.tile([128, G4], F32, tag=f"wh{k}", name=f"wh{k}") for k in range(NKT)]
            for k in range(NKT):
                nc.sync.dma_start(wx_sb[k], wxT.ap()[128 * k : 128 * (k + 1), :])
                nc.sync.dma_start(wf_sb[k], wfT.ap()[128 * k : 128 * (k + 1), :])
                nc.sync.dma_start(wh_sb[k], whT.ap()[128 * k : 128 * (k + 1), :])

            # ---------------- phase 1: precompute ----------------
            with (
                tc.tile_pool(name="gather", bufs=3) as gpool,
                tc.tile_pool(name="xt", bufs=2) as xtpool,
                tc.tile_pool(name="sbout", bufs=3) as opool,
                tc.tile_pool(name="zf", bufs=6) as zfpool,
                tc.tile_pool(name="tp_ps", bufs=2, space="PSUM") as tppool,
                tc.tile_pool(name="mm_ps", bufs=4, space="PSUM") as mmpool,
            ):
                for c in range(NCHUNK):
                    xT = [xtpool.tile([128, 512], F32, tag=f"xT{k}", name=f"xT{k}") for k in range(NKT)]
                    fpT = [xtpool.tile([128, 512], F32, tag=f"fpT{k}", name=f"fpT{k}") for k in range(NKT)]
                    for j in range(CHUNK_TT):
                        jj = CHUNK_TT * c + j
                        xg = gpool.tile([128, H], F32, tag="xg")
                        nc.gpsimd.indirect_dma_start(
                            out=xg[:], out_offset=None, in_=wte.ap(),
                            in_offset=bass.IndirectOffsetOnAxis(ap=ix_sb[:, jj : jj + 1], axis=0),
                        )
                        fg = gpool.tile([128, H], F32, tag="fg")
                        nc.gpsimd.indirect_dma_start(
                            out=fg[:], out_offset=None, in_=wfe.ap(),
                            in_offset=bass.IndirectOffsetOnAxis(ap=if_sb[:, jj : jj + 1], axis=0),
                        )
                        pg = gpool.tile([128, H], F32, tag="pg")
                        nc.gpsimd.indirect_dma_start(
                            out=pg[:], out_offset=None, in_=wpe.ap(),
                            in_offset=bass.IndirectOffsetOnAxis(ap=ip_sb[:, jj : jj + 1], axis=0),
                        )
                        rg = gpool.tile([128, H], F32, tag="rg")
                        nc.gpsimd.indirect_dma_start(
                            out=rg[:], out_offset=None, in_=wpe.ap(),
                            in_offset=bass.IndirectOffsetOnAxis(ap=ir_sb[:, jj : jj + 1], axis=0),
                        )
                        fps = gpool.tile([128, H], F32, tag="fps")
                        nc.vector.tensor_add(fps, fg, pg)
                        nc.vector.tensor_add(fps, fps, rg)
                        nc.sync.dma_start(fp_out.ap()[jj * 128 : (jj + 1) * 128, :], fps)
                        for k in range(NKT):
                            ps = tppool.tile([128, 128], F32, tag="tp")
                            nc.tensor.transpose(ps[:], xg[:, 128 * k : 128 * (k + 1)], ident[:])
                            nc.vector.tensor_copy(xT[k][:, 128 * j : 128 * (j + 1)], ps)
                            ps2 = tppool.tile([128, 128], F32, tag="tp")
                            nc.tensor.transpose(ps2[:], fps[:, 128 * k : 128 * (k + 1)], ident[:])
                            nc.vector.tensor_copy(fpT[k][:, 128 * j : 128 * (j + 1)], ps2)
                    # gx = x @ wx.T + b   -> [gate tiles, 512 tokens]
                    for gi in range(NGT):
                        ps = mmpool.tile([128, 512], F32, tag="mm")
                        for k in range(NKT):
                            nc.tensor.matmul(
                                ps[:], wx_sb[k][:, 128 * gi : 128 * (gi + 1)], xT[k][:],
                                start=(k == 0), stop=(k == NKT - 1),
                            )
                        gxsb = opool.tile([128, 512], F32, tag="gx")
                        nc.scalar.activation(gxsb, ps, AF.Identity, bias=bg_sb[:, gi : gi + 1])
                        nc.sync.dma_start(
                            gx_d.ap()[:, gi * NTOK + 512 * c : gi * NTOK + 512 * (c + 1)], gxsb
                        )
                    # zf = field_pos @ wf.T + bf ; lz = sigmoid(zf_lo)*tanh(zf_hi)
                    zf_t = {}
                    for gi in range(NZT):
                        ps = mmpool.tile([128, 512], F32, tag="mm")
                        for k in range(NKT):
                            nc.tensor.matmul(
                                ps[:], wf_sb[k][:, 128 * gi : 128 * (gi + 1)], fpT[k][:],
                                start=(k == 0), stop=(k == NKT - 1),
                            )
                        z = zfpool.tile([128, 512], F32, tag="zf")
                        nc.scalar.activation(z, ps, AF.Identity, bias=bfg_sb[:, gi : gi + 1])
                        zf_t[gi] = z
                    for j2 in range(NKT):
                        lsb = opool.tile([128, 512], F32, tag="lsb")
                        nc.scalar.activation(lsb, zf_t[j2], AF.Sigmoid)
                        zsb = opool.tile([128, 512], F32, tag="zsb")
                        nc.scalar.activation(zsb, zf_t[NKT + j2], AF.Tanh)
                        lz = opool.tile([128, 512], F32, tag="lz")
                        nc.vector.tensor_mul(lz, lsb, zsb)
                        nc.sync.dma_start(
                            lz_d.ap()[:, j2 * NTOK + 512 * c : j2 * NTOK + 512 * (c + 1)], lz
                        )

            # ---------------- phase 2: scan ----------------
            gx3 = gx_d.ap().rearrange("p (g u) -> p g u", g=NGT)
            lz3 = lz_d.ap().rearrange("p (j u) -> p j u", j=NKT)
            with (
                tc.tile_pool(name="gxc", bufs=2) as gxcp,
                tc.tile_pool(name="lzc", bufs=2) as lzcp,
                tc.tile_pool(name="hpool", bufs=4) as hpool,
                tc.tile_pool(name="cpool", bufs=3) as cpool,
                tc.tile_pool(name="gsb", bufs=2) as gsbp,
                tc.tile_pool(name="acts", bufs=3) as actp,
                tc.tile_pool(name="g_ps", bufs=2, space="PSUM") as gpsp,
            ):
                h = hpool.tile([128, 32], F32, tag="h")
                nc.vector.memset(h[:], 0.0)
                cprev = cpool.tile([128, 32], F32, tag="c")
                nc.vector.memset(cprev[:], 0.0)

                nchunks = (steps + SCAN_CH - 1) // SCAN_CH
                gxc_t, lzc_t = {}, {}

                def fetch(ci):
                    g = gxcp.tile([128, NGT * SCAN_CH * 8], F32, tag="gxc")
                    nc.sync.dma_start(
                        g[:].rearrange("p (g u) -> p g u", g=NGT),
                        gx3[:, :, SCAN_CH * 8 * ci : SCAN_CH * 8 * (ci + 1)],
                    )
                    gxc_t[ci] = g
                    l = lzcp.tile([128, NKT * SCAN_CH * 8], F32, tag="lzc")
                    nc.sync.dma_start(
                        l[:].rearrange("p (j u) -> p j u", j=NKT),
                        lz3[:, :, SCAN_CH * 8 * ci : SCAN_CH * 8 * (ci + 1)],
                    )
                    lzc_t[ci] = l

                fetch(0)
                for t in range(steps):
                    ci, tr = divmod(t, SCAN_CH)
                    if tr == 0 and ci + 1 < nchunks:
                        fetch(ci + 1)
                    gxc, lzc = gxc_t[ci], lzc_t[ci]

                    gps = gpsp.tile([128, 128], F32, tag="g")
                    for gi in range(NGT):
                        o = gps[:, 8 * gi : 8 * (gi + 1)]
                        for k in range(NKT):
                            nc.tensor.matmul(
                                o, wh_sb[k][:, 128 * gi : 128 * (gi + 1)], h[:, 8 * k : 8 * (k + 1)],
                                start=(k == 0), stop=(k == NKT - 1),
                            )
                    gsb = gsbp.tile([128, 128], F32, tag="gsb")
                    nc.vector.tensor_add(
                        gsb[:].rearrange("p (g b) -> p g b", g=NGT),
                        gps[:].rearrange("p (g b) -> p g b", g=NGT),
                        gxc[:].rearrange("p (g u) -> p g u", g=NGT)[:, :, 8 * tr : 8 * tr + 8],
                    )
                    sig = actp.tile([128, 96], F32, tag="sig")
                    nc.scalar.activation(sig, gsb[:, 0:96], AF.Sigmoid)
                    tnh = actp.tile([128, 32], F32, tag="tnh")
                    nc.scalar.activation(tnh, gsb[:, 96:128], AF.Tanh)
                    tmp = actp.tile([128, 32], F32, tag="tmp")
                    nc.vector.tensor_mul(tmp, sig[:, 0:32], tnh)       # i * c_hat
                    cnew = cpool.tile([128, 32], F32, tag="c")
                    nc.vector.tensor_mul(cnew, sig[:, 32:64], cprev)   # f * c
                    nc.vector.tensor_add(cnew, cnew, tmp)
                    nc.vector.tensor_add(
                        cnew[:].rearrange("p (j b) -> p j b", j=NKT),
                        cnew[:].rearrange("p (j b) -> p j b", j=NKT),
                        lzc[:].rearrange("p (j u) -> p j u", j=NKT)[:, :, 8 * tr : 8 * tr + 8],
                    )
                    tcn = actp.tile([128, 32], F32, tag="tcn")
                    nc.scalar.activation(tcn, cnew, AF.Tanh)
                    hnew = hpool.tile([128, 32], F32, tag="h")
                    nc.vector.tensor_mul(hnew, sig[:, 64:96], tcn)     # o * tanh(c)
                    nc.sync.dma_start(hs_out.ap()[:, 32 * t : 32 * (t + 1)], hnew)
                    h = hnew
                    cprev = cnew
                nc.sync.dma_start(c_out.ap(), cprev)

    nsplit = _split_multiwaits(nc)
    return nc


_CACHE = {}


def _get_nc():
    if "nc" not in _CACHE:
        _CACHE["nc"] = _build()
    return _CACHE["nc"]


def _run_timed(nc, in_maps, n_iters=3):
    """Replicates bass2jax.run_bass_via_pjrt but keeps inputs device-resident
    and times repeated executions (min over iters).  Dev/profiling only."""
    import time as _time

    import jax
    from jax.sharding import Mesh, PartitionSpec
    from jax.experimental.shard_map import shard_map
    from concourse import bass2jax, mybir as mb

    bass2jax.install_neuronx_cc_hook()
    n_cores = len(in_maps)
    partition_name = nc.partition_id_tensor.name if nc.partition_id_tensor else None
    in_names, out_names, out_avals, zero_shapes = [], [], [], []
    for alloc in nc.m.functions[0].allocations:
        if not isinstance(alloc, mb.MemoryLocationSet):
            continue
        name = alloc.memorylocations[0].name
        if alloc.kind == "ExternalInput":
            if name != partition_name:
                in_names.append(name)
        elif alloc.kind == "ExternalOutput":
            out_names.append(name)
            shape = tuple(alloc.tensor_shape)
            dtype = mb.dt.np(alloc.dtype)
            out_avals.append(jax.core.ShapedArray(shape, dtype))
            zero_shapes.append((shape, dtype))
    n_params = len(in_names)
    n_outs = len(out_avals)
    all_names = in_names + out_names
    if partition_name is not None:
        all_names = all_names + [partition_name]

    def _body(*args):
        operands = list(args)
        if partition_name is not None:
            operands.append(bass2jax.partition_id_tensor())
        outs = bass2jax._bass_exec_p.bind(
            *operands,
            out_avals=tuple(out_avals),
            in_names=tuple(all_names),
            out_names=tuple(out_names),
            lowering_input_output_aliases=(),
            sim_require_finite=True,
            sim_require_nnan=True,
            nc=nc,
        )
        return tuple(outs)

    devices = jax.devices()[:n_cores]
    mesh = Mesh(np.asarray(devices), ("core",))
    donate = tuple(range(n_params, n_params + n_outs))
    sharded = jax.jit(
        shard_map(
            _body,
            mesh=mesh,
            in_specs=(PartitionSpec("core"),) * (n_params + n_outs),
            out_specs=(PartitionSpec("core"),) * n_outs,
            check_rep=False,
        ),
        donate_argnums=donate,
        keep_unused=True,
    )
    import jax.numpy as jnp
    from jax.sharding import NamedSharding

    shard = NamedSharding(mesh, PartitionSpec("core"))
    concat_in = [
        jax.device_put(
            np.concatenate([np.asarray(in_maps[c][n]) for c in range(n_cores)], axis=0),
            shard,
        )
        for n in in_names
    ]
    mkzeros = jax.jit(
        lambda: tuple(
            jnp.zeros((n_cores * s[0], *s[1:]), d) for s, d in zero_shapes
        ),
        out_shardings=(shard,) * n_outs,
    )
    times = []
    out_arrs = None
    for _ in range(n_iters):
        zs = jax.block_until_ready(mkzeros())
        t0 = _time.perf_counter()
        out_arrs = jax.block_until_ready(sharded(*concat_in, *zs))
        times.append(_time.perf_counter() - t0)
    results = [
        {
            name: np.asarray(out_arrs[i]).reshape(n_cores, *out_avals[i].shape)[c]
            for i, name in enumerate(out_names)
        }
        for c in range(n_cores)
    ]
    return results, times


def _prep_idx(a):
    # [BL, S] batch-slice -> time-major token order tok = s*BL + b, tiled
    # [128, NTT] with tile j in column j: idx[p, j] = tok_list[j*128 + p]
    arr = np.ascontiguousarray(np.asarray(a).T).reshape(-1)  # tok = s*BL + b
    return np.ascontiguousarray(arr.reshape(NTT, 128).T).astype(np.int32)


def kernel(inputs, fields, pos, rpos, wte, wfe, wpe, w, b, wf, bf):
    inputs, fields, pos, rpos = (np.asarray(a) for a in (inputs, fields, pos, rpos))
    wte, wfe, wpe, w, b, wf, bf = (
        np.asarray(a, dtype=np.float32) for a in (wte, wfe, wpe, w, b, wf, bf)
    )
    Hh = H
    wxT = np.ascontiguousarray(w[:, :Hh].T)          # [512, 2048]
    whT = np.ascontiguousarray(w[:, Hh:].T)          # [512, 2048]
    wfT = np.ascontiguousarray(wf.T)                 # [512, 1024]
    bg = np.ascontiguousarray(b[:, 0].reshape(NGT, 128).T)    # [128, 16]
    bfg = np.ascontiguousarray(bf[:, 0].reshape(NZT, 128).T)  # [128, 8]

    in_maps = []
    for c in range(NCORES):
        sl = slice(c * BL, (c + 1) * BL)
        in_maps.append(
            {
                "idx_x": _prep_idx(inputs[sl]),
                "idx_f": _prep_idx(fields[sl]),
                "idx_p": _prep_idx(pos[sl]),
                "idx_r": _prep_idx(rpos[sl]),
                "wte": wte, "wfe": wfe, "wpe": wpe,
                "wxT": wxT, "whT": whT, "wfT": wfT,
                "bg": bg, "bfg": bfg,
            }
        )

    nc = _get_nc()
    if bool(int(os.environ.get("FGATE_TRACE", "0"))):
        results, times = _run_timed(nc, in_maps, n_iters=4)
        _CACHE["last_exec_time_ns"] = int(min(times) * 1e9)
        _CACHE["last_times"] = times
    else:
        res = run_bass_kernel_spmd(nc, in_maps, list(range(NCORES)))
        _CACHE["last_exec_time_ns"] = res.exec_time_ns
        results = res.results

    steps = _STEPS
    hs = np.empty((steps, B, H), np.float32)
    fp = np.empty((S, B, H), np.float32)
    cfin = np.empty((B, H), np.float32)
    for c in range(NCORES):
        r = results[c]
        sl = slice(c * BL, (c + 1) * BL)
        # hs_out: [p, 32*t + 8k + b] -> hs[t, b, 128k+p]
        hs_r = r["hs_out"].reshape(128, S, NKT, BL)[:, :steps]
        hs[:, sl, :] = hs_r.transpose(1, 3, 2, 0).reshape(steps, BL, H)
        fp[:, sl, :] = r["fp_out"].reshape(S, BL, H)
        cfin[sl] = r["c_out"].reshape(128, NKT, BL).transpose(2, 1, 0).reshape(BL, H)
    hfin = hs[-1]
    return hs, fp, (hfin[None], cfin[None])


# revision 13
# speedup vs baseline: 17.4294x; 17.4294x over previous
"""Trainium2 Bass kernel for nn_FGateEncoder (LSTM-style recurrence with field gate).

Strategy: data-parallel over batch across 8 NeuronCores (8 sequences per core,
weights replicated). Per core:
  1. Gather wte/wfe/wpe embedding rows on-device via indirect DMA
     (token-major tiles [128 tokens, 512]).
  2. PE-transpose to [H-on-partitions, tokens] and run the big time-independent
     matmuls: gx = x @ wx.T + b  and  zf = field_pos @ wf.T + bf, then
     lz = sigmoid(zf[:H]) * tanh(zf[H:]).  Results spill to DRAM scratch in a
     time-major layout the scan can stream.
  3. Sequential 512-step scan, everything laid out [feature-on-partitions,
     batch(8) on free dim]: per step 64 accumulating matmuls
     (g[128gate,8b] += whT_tile.T @ h_chunk), sigmoid/tanh on the scalar
     engine, c/h updates on the vector engine.  No transposes on the critical
     path; h chunks feed the next step's matmuls directly.
Host only reorders/slices inputs and reassembles outputs.
"""

import os

import numpy as np

import concourse.bass as bass
import concourse.mybir as mybir
import concourse.tile as tile
from concourse.bass_utils import run_bass_kernel_spmd
from concourse.masks import make_identity

F32 = mybir.dt.float32
F16 = mybir.dt.float16
SCAN_DT = F16
I32 = mybir.dt.int32
AF = mybir.ActivationFunctionType

B, S, H = 64, 512, 512
V, FV, PV = 50257, 2000, 1024
NCORES = 8
BL = B // NCORES          # 8 sequences per core
NTOK = S * BL             # 4096 tokens per core
NTT = NTOK // 128         # 32 token tiles
CHUNK_TT = 4              # token tiles per precompute chunk (512 tokens)
NCHUNK = NTT // CHUNK_TT  # 8
SCAN_CH = 16              # scan steps per gx/lz stream chunk
G4 = 4 * H                # 2048 gate dims
NGT = G4 // 128           # 16 gate tiles
NZT = 2 * H // 128        # 8 zf tiles
NKT = H // 128            # 4 contraction tiles

_STEPS = int(os.environ.get("FGATE_DEV_STEPS", str(S)))


def _split_multiwaits(nc):
    """walrus in this container rejects >1 sync-wait per instruction; Tile's
    kernel-tail drain accumulates several.  Split extras into standalone
    sequencer waits placed immediately before the offending instruction."""
    n = 0
    for f in nc.m.functions:
        for blk in f.blocks:
            out = []
            for inst in blk.instructions:
                si = inst.sync_info
                if si is not None and si.on_wait is not None and len(si.on_wait) > 1:
                    waits = list(si.on_wait)
                    for j, w in enumerate(waits[:-1]):
                        out.append(
                            mybir.InstEventSemaphore(
                                name=f"{inst.name}_splitwait_{j}",
                                engine=inst.engine,
                                ins=[],
                                outs=[],
                                sync_info=mybir.SyncInfo(on_wait=[w], on_update=[]),
                            )
                        )
                        n += 1
                    inst.sync_info = mybir.SyncInfo(
                        on_wait=[waits[-1]], on_update=si.on_update
                    )
                out.append(inst)
            blk.instructions = out
    return n


def _build(steps=_STEPS):
    nc = bass.Bass("TRN2", target_bir_lowering=False, debug=False, num_devices=NCORES)

    # ---- I/O ----
    idx_x = nc.dram_tensor("idx_x", [128, NTT], I32, kind="ExternalInput")
    idx_f = nc.dram_tensor("idx_f", [128, NTT], I32, kind="ExternalInput")
    idx_p = nc.dram_tensor("idx_p", [128, NTT], I32, kind="ExternalInput")
    idx_r = nc.dram_tensor("idx_r", [128, NTT], I32, kind="ExternalInput")
    wte = nc.dram_tensor("wte", [V, H], F32, kind="ExternalInput")
    wfe = nc.dram_tensor("wfe", [FV, H], F32, kind="ExternalInput")
    wpe = nc.dram_tensor("wpe", [PV, H], F32, kind="ExternalInput")
    wxT = nc.dram_tensor("wxT", [H, G4], F16, kind="ExternalInput")
    whT = nc.dram_tensor("whT", [H, G4], SCAN_DT, kind="ExternalInput")
    wfT = nc.dram_tensor("wfT", [H, 2 * H], F16, kind="ExternalInput")
    bg = nc.dram_tensor("bg", [128, NGT], F32, kind="ExternalInput")
    bfg = nc.dram_tensor("bfg", [128, NZT], F32, kind="ExternalInput")

    fp_out = nc.dram_tensor("fp_out", [NTOK, H], F32, kind="ExternalOutput")
    hs_out = nc.dram_tensor("hs_out", [128, S * 32], F32, kind="ExternalOutput")
    c_out = nc.dram_tensor("c_out", [128, 32], F32, kind="ExternalOutput")

    # DRAM scratch, time-major for the scan:
    #   gx_d col layout: gi*4096 + t*8 + b      (per partition p = gate 128*gi+p)
    #   lz_d col layout: j*4096 + t*8 + b       (per partition p = h-dim 128*j+p)
    gx_d = nc.dram_tensor("gx_scratch", [128, NGT * NTOK], F32)
    lz_d = nc.dram_tensor("lz_scratch", [128, NKT * NTOK], F32)

    with tile.TileContext(nc) as tc:
        with (
            tc.tile_pool(name="const", bufs=1) as constp,
            tc.tile_pool(name="weights", bufs=1) as wpool,
        ):
            ident = constp.tile([128, 128], F32)
            make_identity(nc, ident[:])
            bg_sb = constp.tile([128, NGT], F32)
            nc.sync.dma_start(bg_sb, bg.ap())
            bfg_sb = constp.tile([128, NZT], F32)
            nc.sync.dma_start(bfg_sb, bfg.ap())
            ix_sb = constp.tile([128, NTT], I32)
            nc.sync.dma_start(ix_sb, idx_x.ap())
            if_sb = constp.tile([128, NTT], I32)
            nc.sync.dma_start(if_sb, idx_f.ap())
            ip_sb = constp.tile([128, NTT], I32)
            nc.sync.dma_start(ip_sb, idx_p.ap())
            ir_sb = constp.tile([128, NTT], I32)
            nc.sync.dma_start(ir_sb, idx_r.ap())

            wx_sb = [wpool.tile([128, G4], F16, tag=f"wx{k}", name=f"wx{k}") for k in range(NKT)]
            wf_sb = [wpool.tile([128, 2 * H], F16, tag=f"wf{k}", name=f"wfw{k}") for k in range(NKT)]
            wh_sb = [wpool.tile([128, G4], SCAN_DT, tag=f"wh{k}", name=f"wh{k}") for k in range(NKT)]
            for k in range(NKT):
                nc.sync.dma_start(wx_sb[k], wxT.ap()[128 * k : 128 * (k + 1), :])
                nc.sync.dma_start(wf_sb[k], wfT.ap()[128 * k : 128 * (k + 1), :])
                nc.sync.dma_start(wh_sb[k], whT.ap()[128 * k : 128 * (k + 1), :])

            # ---------------- phase 1: precompute ----------------
            with (
                tc.tile_pool(name="gather", bufs=3) as gpool,
                tc.tile_pool(name="xt", bufs=2) as xtpool,
                tc.tile_pool(name="sbout", bufs=3) as opool,
                tc.tile_pool(name="zf", bufs=6) as zfpool,
                tc.tile_pool(name="tp_ps", bufs=2, space="PSUM") as tppool,
                tc.tile_pool(name="mm_ps", bufs=4, space="PSUM") as mmpool,
            ):
                for c in range(NCHUNK):
                    xT = [xtpool.tile([128, 512], F16, tag=f"xT{k}", name=f"xT{k}") for k in range(NKT)]
                    fpT = [xtpool.tile([128, 512], F16, tag=f"fpT{k}", name=f"fpT{k}") for k in range(NKT)]
                    for j in range(CHUNK_TT):
                        jj = CHUNK_TT * c + j
                        xg = gpool.tile([128, H], F32, tag="xg")
                        nc.gpsimd.indirect_dma_start(
                            out=xg[:], out_offset=None, in_=wte.ap(),
                            in_offset=bass.IndirectOffsetOnAxis(ap=ix_sb[:, jj : jj + 1], axis=0),
                        )
                        fg = gpool.tile([128, H], F32, tag="fg")
                        nc.gpsimd.indirect_dma_start(
                            out=fg[:], out_offset=None, in_=wfe.ap(),
                            in_offset=bass.IndirectOffsetOnAxis(ap=if_sb[:, jj : jj + 1], axis=0),
                        )
                        pg = gpool.tile([128, H], F32, tag="pg")
                        nc.gpsimd.indirect_dma_start(
                            out=pg[:], out_offset=None, in_=wpe.ap(),
                            in_offset=bass.IndirectOffsetOnAxis(ap=ip_sb[:, jj : jj + 1], axis=0),
                        )
                        rg = gpool.tile([128, H], F32, tag="rg")
                        nc.gpsimd.indirect_dma_start(
                            out=rg[:], out_offset=None, in_=wpe.ap(),
                            in_offset=bass.IndirectOffsetOnAxis(ap=ir_sb[:, jj : jj + 1], axis=0),
                        )
                        fps = gpool.tile([128, H], F32, tag="fps")
                        nc.vector.tensor_add(fps, fg, pg)
                        nc.vector.tensor_add(fps, fps, rg)
                        nc.sync.dma_start(fp_out.ap()[jj * 128 : (jj + 1) * 128, :], fps)
                        for k in range(NKT):
                            ps = tppool.tile([128, 128], F32, tag="tp")
                            nc.tensor.transpose(ps[:], xg[:, 128 * k : 128 * (k + 1)], ident[:])
                            nc.vector.tensor_copy(xT[k][:, 128 * j : 128 * (j + 1)], ps)
                            ps2 = tppool.tile([128, 128], F32, tag="tp")
                            nc.tensor.transpose(ps2[:], fps[:, 128 * k : 128 * (k + 1)], ident[:])
                            nc.vector.tensor_copy(fpT[k][:, 128 * j : 128 * (j + 1)], ps2)
                    # gx = x @ wx.T + b   -> [gate tiles, 512 tokens]
                    for gi in range(NGT):
                        ps = mmpool.tile([128, 512], F32, tag="mm")
                        for k in range(NKT):
                            nc.tensor.matmul(
                                ps[:], wx_sb[k][:, 128 * gi : 128 * (gi + 1)], xT[k][:],
                                start=(k == 0), stop=(k == NKT - 1),
                            )
                        gxsb = opool.tile([128, 512], F32, tag="gx")
                        nc.scalar.activation(gxsb, ps, AF.Identity, bias=bg_sb[:, gi : gi + 1])
                        nc.sync.dma_start(
                            gx_d.ap()[:, gi * NTOK + 512 * c : gi * NTOK + 512 * (c + 1)], gxsb
                        )
                    # zf = field_pos @ wf.T + bf ; lz = sigmoid(zf_lo)*tanh(zf_hi)
                    zf_t = {}
                    for gi in range(NZT):
                        ps = mmpool.tile([128, 512], F32, tag="mm")
                        for k in range(NKT):
                            nc.tensor.matmul(
                                ps[:], wf_sb[k][:, 128 * gi : 128 * (gi + 1)], fpT[k][:],
                                start=(k == 0), stop=(k == NKT - 1),
                            )
                        z = zfpool.tile([128, 512], F32, tag="zf")
                        nc.scalar.activation(z, ps, AF.Identity, bias=bfg_sb[:, gi : gi + 1])
                        zf_t[gi] = z
                    for j2 in range(NKT):
                        lsb = opool.tile([128, 512], F32, tag="lsb")
                        nc.scalar.activation(lsb, zf_t[j2], AF.Sigmoid)
                        zsb = opool.tile([128, 512], F32, tag="zsb")
                        nc.scalar.activation(zsb, zf_t[NKT + j2], AF.Tanh)
                        lz = opool.tile([128, 512], F32, tag="lz")
                        nc.vector.tensor_mul(lz, lsb, zsb)
                        nc.sync.dma_start(
                            lz_d.ap()[:, j2 * NTOK + 512 * c : j2 * NTOK + 512 * (c + 1)], lz
                        )

            # ---------------- phase 2: scan ----------------
            gx3 = gx_d.ap().rearrange("p (g u) -> p g u", g=NGT)
            lz3 = lz_d.ap().rearrange("p (j u) -> p j u", j=NKT)
            with (
                tc.tile_pool(name="gxc", bufs=2) as gxcp,
                tc.tile_pool(name="lzc", bufs=2) as lzcp,
                tc.tile_pool(name="hpool", bufs=4) as hpool,
                tc.tile_pool(name="cpool", bufs=3) as cpool,
                tc.tile_pool(name="gsb", bufs=2) as gsbp,
                tc.tile_pool(name="acts", bufs=3) as actp,
                tc.tile_pool(name="g_ps", bufs=2, space="PSUM") as gpsp,
            ):
                cprev = cpool.tile([128, 32], F32, tag="c")
                nc.vector.memset(cprev[:], 0.0)

                nchunks = (steps + SCAN_CH - 1) // SCAN_CH
                gxc_t, lzc_t = {}, {}

                def fetch(ci):
                    g = gxcp.tile([128, NGT * SCAN_CH * 8], F32, tag="gxc")
                    nc.sync.dma_start(
                        g[:].rearrange("p (g u) -> p g u", g=NGT),
                        gx3[:, :, SCAN_CH * 8 * ci : SCAN_CH * 8 * (ci + 1)],
                    )
                    gxc_t[ci] = g
                    l = lzcp.tile([128, NKT * SCAN_CH * 8], F32, tag="lzc")
                    nc.sync.dma_start(
                        l[:].rearrange("p (j u) -> p j u", j=NKT),
                        lz3[:, :, SCAN_CH * 8 * ci : SCAN_CH * 8 * (ci + 1)],
                    )
                    lzc_t[ci] = l

                fetch(0)
                h16 = hpool.tile([128, 32], SCAN_DT, tag="h16", name="h16_init")
                nc.vector.memset(h16[:], 0.0)
                for t in range(steps):
                    ci, tr = divmod(t, SCAN_CH)
                    if tr == 0 and ci + 1 < nchunks:
                        fetch(ci + 1)
                    gxc, lzc = gxc_t[ci], lzc_t[ci]
                    gxv = gxc[:].rearrange("p (m j u) -> p m j u", m=4, j=4)
                    lzv = lzc[:].rearrange("p (j u) -> p j u", j=NKT)

                    gps = gpsp.tile([128, 128], F32, tag="g")
                    gpv = gps[:].rearrange("p (m j b) -> p m j b", m=4, j=4)
                    gsb = gsbp.tile([128, 128], F32, tag="gsb")
                    gsv = gsb[:].rearrange("p (m j b) -> p m j b", m=4, j=4)
                    cnew = cpool.tile([128, 32], F32, tag="c")
                    hnew = hpool.tile([128, 32], F32, tag="h")
                    h16n = hpool.tile([128, 32], SCAN_DT, tag="h16")
                    for j in range(4):
                        for m in range(4):
                            gi = 4 * m + j
                            o = gps[:, 8 * gi : 8 * (gi + 1)]
                            for k in range(NKT):
                                nc.tensor.matmul(
                                    o, wh_sb[k][:, 128 * gi : 128 * (gi + 1)],
                                    h16[:, 8 * k : 8 * (k + 1)],
                                    start=(k == 0), stop=(k == NKT - 1),
                                )
                        # tail for h-dim group j (gates gi = 4m + j)
                        nc.vector.tensor_add(
                            gsv[:, :, j : j + 1, :], gpv[:, :, j : j + 1, :],
                            gxv[:, :, j : j + 1, 8 * tr : 8 * tr + 8],
                        )
                        sj = actp.tile([128, 24], F32, tag="sig")
                        nc.scalar.activation(sj, gsv[:, 0:3, j : j + 1, :], AF.Sigmoid)
                        tj = actp.tile([128, 8], F32, tag="tnh")
                        nc.scalar.activation(tj, gsv[:, 3:4, j : j + 1, :], AF.Tanh)
                        cj = cnew[:, 8 * j : 8 * (j + 1)]
                        nc.vector.tensor_mul(cj, sj[:, 8:16], cprev[:, 8 * j : 8 * (j + 1)])
                        tmpj = actp.tile([128, 8], F32, tag="tmp")
                        nc.vector.tensor_mul(tmpj, sj[:, 0:8], tj)
                        nc.vector.tensor_add(tmpj, tmpj, lzv[:, j : j + 1, 8 * tr : 8 * tr + 8])
                        nc.vector.tensor_add(cj, cj, tmpj)
                        tcj = actp.tile([128, 8], F32, tag="tcn")
                        nc.scalar.activation(tcj, cj, AF.Tanh)
                        hj = hnew[:, 8 * j : 8 * (j + 1)]
                        nc.vector.tensor_mul(hj, sj[:, 16:24], tcj)
                        nc.vector.tensor_copy(h16n[:, 8 * j : 8 * (j + 1)], hj)
                    nc.sync.dma_start(hs_out.ap()[:, 32 * t : 32 * (t + 1)], hnew)
                    h16 = h16n
                    cprev = cnew
                nc.sync.dma_start(c_out.ap(), cprev)

    nsplit = _split_multiwaits(nc)
    return nc


_CACHE = {}


def _get_nc():
    if "nc" not in _CACHE:
        _CACHE["nc"] = _build()
    return _CACHE["nc"]


def _run_timed(nc, in_maps, n_iters=3):
    """Replicates bass2jax.run_bass_via_pjrt but keeps inputs device-resident
    and times repeated executions (min over iters).  Dev/profiling only."""
    import time as _time

    import jax
    from jax.sharding import Mesh, PartitionSpec
    from jax.experimental.shard_map import shard_map
    from concourse import bass2jax, mybir as mb

    bass2jax.install_neuronx_cc_hook()
    n_cores = len(in_maps)
    partition_name = nc.partition_id_tensor.name if nc.partition_id_tensor else None
    in_names, out_names, out_avals, zero_shapes = [], [], [], []
    for alloc in nc.m.functions[0].allocations:
        if not isinstance(alloc, mb.MemoryLocationSet):
            continue
        name = alloc.memorylocations[0].name
        if alloc.kind == "ExternalInput":
            if name != partition_name:
                in_names.append(name)
        elif alloc.kind == "ExternalOutput":
            out_names.append(name)
            shape = tuple(alloc.tensor_shape)
            dtype = mb.dt.np(alloc.dtype)
            out_avals.append(jax.core.ShapedArray(shape, dtype))
            zero_shapes.append((shape, dtype))
    n_params = len(in_names)
    n_outs = len(out_avals)
    all_names = in_names + out_names
    if partition_name is not None:
        all_names = all_names + [partition_name]

    def _body(*args):
        operands = list(args)
        if partition_name is not None:
            operands.append(bass2jax.partition_id_tensor())
        outs = bass2jax._bass_exec_p.bind(
            *operands,
            out_avals=tuple(out_avals),
            in_names=tuple(all_names),
            out_names=tuple(out_names),
            lowering_input_output_aliases=(),
            sim_require_finite=True,
            sim_require_nnan=True,
            nc=nc,
        )
        return tuple(outs)

    devices = jax.devices()[:n_cores]
    mesh = Mesh(np.asarray(devices), ("core",))
    donate = tuple(range(n_params, n_params + n_outs))
    sharded = jax.jit(
        shard_map(
            _body,
            mesh=mesh,
            in_specs=(PartitionSpec("core"),) * (n_params + n_outs),
            out_specs=(PartitionSpec("core"),) * n_outs,
            check_rep=False,
        ),
        donate_argnums=donate,
        keep_unused=True,
    )
    import jax.numpy as jnp
    from jax.sharding import NamedSharding

    shard = NamedSharding(mesh, PartitionSpec("core"))
    concat_in = [
        jax.device_put(
            np.concatenate([np.asarray(in_maps[c][n]) for c in range(n_cores)], axis=0),
            shard,
        )
        for n in in_names
    ]
    mkzeros = jax.jit(
        lambda: tuple(
            jnp.zeros((n_cores * s[0], *s[1:]), d) for s, d in zero_shapes
        ),
        out_shardings=(shard,) * n_outs,
    )
    times = []
    out_arrs = None
    for _ in range(n_iters):
        zs = jax.block_until_ready(mkzeros())
        t0 = _time.perf_counter()
        out_arrs = jax.block_until_ready(sharded(*concat_in, *zs))
        times.append(_time.perf_counter() - t0)
    results = [
        {
            name: np.asarray(out_arrs[i]).reshape(n_cores, *out_avals[i].shape)[c]
            for i, name in enumerate(out_names)
        }
        for c in range(n_cores)
    ]
    return results, times


def _prep_idx(a):
    # [BL, S] batch-slice -> time-major token order tok = s*BL + b, tiled
    # [128, NTT] with tile j in column j: idx[p, j] = tok_list[j*128 + p]
    arr = np.ascontiguousarray(np.asarray(a).T).reshape(-1)  # tok = s*BL + b
    return np.ascontiguousarray(arr.reshape(NTT, 128).T).astype(np.int32)


def kernel(inputs, fields, pos, rpos, wte, wfe, wpe, w, b, wf, bf):
    inputs, fields, pos, rpos = (np.asarray(a) for a in (inputs, fields, pos, rpos))
    wte, wfe, wpe, w, b, wf, bf = (
        np.asarray(a, dtype=np.float32) for a in (wte, wfe, wpe, w, b, wf, bf)
    )
    Hh = H
    wxT = np.ascontiguousarray(w[:, :Hh].T).astype(np.float16)  # [512, 2048]
    whT = np.ascontiguousarray(w[:, Hh:].T).astype(np.float16)  # [512, 2048]
    wfT = np.ascontiguousarray(wf.T).astype(np.float16)  # [512, 1024]
    bg = np.ascontiguousarray(b[:, 0].reshape(NGT, 128).T)    # [128, 16]
    bfg = np.ascontiguousarray(bf[:, 0].reshape(NZT, 128).T)  # [128, 8]

    in_maps = []
    for c in range(NCORES):
        sl = slice(c * BL, (c + 1) * BL)
        in_maps.append(
            {
                "idx_x": _prep_idx(inputs[sl]),
                "idx_f": _prep_idx(fields[sl]),
                "idx_p": _prep_idx(pos[sl]),
                "idx_r": _prep_idx(rpos[sl]),
                "wte": wte, "wfe": wfe, "wpe": wpe,
                "wxT": wxT, "whT": whT, "wfT": wfT,
                "bg": bg, "bfg": bfg,
            }
        )

    nc = _get_nc()
    if bool(int(os.environ.get("FGATE_TRACE", "0"))):
        n_iters = int(os.environ.get("FGATE_ITERS", "12"))
        results, times = _run_timed(nc, in_maps, n_iters=n_iters)
        _CACHE["last_exec_time_ns"] = int(min(times) * 1e9)
        _CACHE["last_times"] = times
    else:
        res = run_bass_kernel_spmd(nc, in_maps, list(range(NCORES)))
        _CACHE["last_exec_time_ns"] = res.exec_time_ns
        results = res.results

    steps = _STEPS
    hs = np.empty((steps, B, H), np.float32)
    fp = np.empty((S, B, H), np.float32)
    cfin = np.empty((B, H), np.float32)
    for c in range(NCORES):
        r = results[c]
        sl = slice(c * BL, (c + 1) * BL)
        # hs_out: [p, 32*t + 8k + b] -> hs[t, b, 128k+p]
        hs_r = r["hs_out"].reshape(128, S, NKT, BL)[:, :steps]
        hs[:, sl, :] = hs_r.transpose(1, 3, 2, 0).reshape(steps, BL, H)
        fp[:, sl, :] = r["fp_out"].reshape(S, BL, H)
        cfin[sl] = r["c_out"].reshape(128, NKT, BL).transpose(2, 1, 0).reshape(BL, H)
    hfin = hs[-1]
    return hs, fp, (hfin[None], cfin[None])
